# revision 1
# baseline (speedup 1.0000x reference)
"""AttnBlock (GroupNorm + single-head spatial attention + residual) on 8 TRN2 cores.

Sharding: core i handles batch b=i//2, query-half h=i%2 (2048 of 4096 spatial
positions). Keys/values span all 4096 positions, computed per-core from the
same batch input — no collectives. The host permutes each core's input so its
query half is always columns [0,2048): attention is permutation-invariant over
keys, so k/v order doesn't matter as long as q/residual/output use the same
order.

Precision: fp16 matmul operands (PE runs 16-bit at 1 cycle/row vs 4 for fp32),
fp32 PSUM accumulation, fp32 softmax stats / GroupNorm / residual. Host folds
1/sqrt(C) into q_w and v_b into the proj bias (softmax rows sum to 1).

DMA discipline: every DMA descriptor has exactly ONE wait slot (ISA
NEURON_ISA_TPB_EVENTS), so no DMA may target a recycled buffer (>=2 deps).
x stays SBUF-resident (loaded once via unique-range DMAs with zero waits) and
output stores carry a single DVE wait.
"""
import sys

for p in ("/opt/trn_rl_repo",):
    if p not in sys.path:
        sys.path.insert(0, p)

import numpy as np

import concourse.bass as bass
import concourse.mybir as mybir
import concourse.tile as tile

B, C, HW = 4, 512, 4096
NQ = HW // 2           # query positions per core
CC = C // 128          # channel chunks
F32 = mybir.dt.float32
F16 = mybir.dt.float16
AX = mybir.AxisListType.X
AF = mybir.ActivationFunctionType


def build_kernel():
    nc = bass.Bass()
    xb = nc.dram_tensor("xb", [C, HW], F32, kind="ExternalInput")
    wq = nc.dram_tensor("wq", [128, CC, C], F16, kind="ExternalInput")
    wk = nc.dram_tensor("wk", [128, CC, C], F16, kind="ExternalInput")
    wv = nc.dram_tensor("wv", [128, CC, C], F16, kind="ExternalInput")
    wp = nc.dram_tensor("wp", [128, CC, C], F16, kind="ExternalInput")
    bq = nc.dram_tensor("bq", [128, CC], F32, kind="ExternalInput")
    bk = nc.dram_tensor("bk", [128, CC], F32, kind="ExternalInput")
    bp = nc.dram_tensor("bp", [128, CC], F32, kind="ExternalInput")
    gw = nc.dram_tensor("gw", [128, CC], F32, kind="ExternalInput")
    gb = nc.dram_tensor("gb", [128, CC], F32, kind="ExternalInput")
    gA = nc.dram_tensor("gA", [128, 8], F16, kind="ExternalInput")
    gB = nc.dram_tensor("gB", [8, 128], F16, kind="ExternalInput")
    eye = nc.dram_tensor("eye", [128, 128], F16, kind="ExternalInput")
    out = nc.dram_tensor("out", [C, NQ], F32, kind="ExternalOutput")

    xv = xb.rearrange("(cc p) n -> p cc n", p=128)      # [128, CC, HW]
    ov = out.rearrange("(cc p) n -> p cc n", p=128)     # [128, CC, NQ]

    with tile.TileContext(nc) as tc:
        ost_full = build_body(nc, tc, xv, ov, wq, wk, wv, wp, bq, bk, bp,
                              gw, gb, gA, gB, eye)
    _legalize_waits(nc)
    sem = nc.alloc_semaphore("st_sem", num=next(nc._free_sem_ids))
    end_sem = nc.alloc_semaphore("end_sem", num=next(nc._free_sem_ids))
    nc.sync.dma_start(out=ov[:, :, :], in_=ost_full[:, :, :]).then_inc(sem, 16)
    nc.sync.wait_ge(sem, 16).then_inc(end_sem, 1)
    return nc


def _legalize_waits(nc):
    """Walrus codegen allows ONE sync wait per ISA instruction (TPB_EVENTS has a
    single wait slot). Tile can emit several (same-engine pipeline hazard +
    cross-engine deps). Split: keep one wait on the instruction, move the rest
    onto engine NoOps inserted immediately before it (same engine queue)."""
    import bass_rust as _br
    used = set()
    for fn in nc.m.functions:
        for blk in fn.blocks:
            for inst in blk.instructions:
                si = inst.sync_info
                if si is not None:
                    for e in list(si.on_wait or []) + list(si.on_update or []):
                        used.add(e.id)
    free_ids = (i for i in range(254, 0, -1) if i not in used)
    nc._free_sem_ids = free_ids
    legal_sems = {}
    for fn in nc.m.functions:
        for blk in fn.blocks:
            out = []
            for inst in blk.instructions:
                si = inst.sync_info
                waits = list(si.on_wait) if si is not None and si.on_wait else []
                if len(waits) > 1:
                    if isinstance(inst, mybir.InstDMACopy):
                        raise RuntimeError(
                            f"DMA {inst.name} has {len(waits)} waits; DMA queues "
                            "cannot be legalized with nops - restructure deps")
                    for w in waits[:-1]:
                        nop = mybir.InstNoOp(
                            name=nc.get_next_instruction_name(),
                            engine=inst.engine,
                            bass_nofuse=True,
                            sync_info=mybir.SyncInfo(on_wait=[w], on_update=[]),
                        )
                        if inst.engine not in legal_sems:
                            legal_sems[inst.engine] = nc.alloc_semaphore(
                                f"legalize_sem_{inst.engine}", num=next(free_ids))
                        _br.then_inc(nop, legal_sems[inst.engine], 1, False)
                        out.append(nop)
                    inst.sync_info = mybir.SyncInfo(
                        on_wait=[waits[-1]], on_update=list(si.on_update or []))
                out.append(inst)
            blk.instructions = out


def build_body(nc, tc, xv, ov, wq, wk, wv, wp, bq, bk, bp, gw, gb, gA, gB, eye):
    import contextlib

    ctx = contextlib.ExitStack()
    with ctx:
        res = ctx.enter_context(tc.tile_pool(name="res", bufs=1))     # resident
        scp = ctx.enter_context(tc.tile_pool(name="scp", bufs=2, space="PSUM"))
        avp = ctx.enter_context(tc.tile_pool(name="avp", bufs=1, space="PSUM"))

        # --- resident tensors ---
        kt = res.tile([128, CC, HW], F16, tag="kt")        # k[c,j]
        vt = res.tile([128, HW // 128, C], F16, tag="vt")  # vT[j,c]
        qt = res.tile([128, CC, NQ], F16, tag="qt")        # q[c,i] (scaled)
        xlo = res.tile([128, CC, NQ], F32, tag="xlo")      # x cols [0,2048)
        twq = res.tile([128, CC, C], F16, tag="twq")
        twk = res.tile([128, CC, C], F16, tag="twk")
        twv = res.tile([128, CC, C], F16, tag="twv")
        twp = res.tile([128, CC, C], F16, tag="twp")
        tbq = res.tile([128, CC], F32, tag="tbq")
        tbk = res.tile([128, CC], F32, tag="tbk")
        tbp = res.tile([128, CC], F32, tag="tbp")
        tgw = res.tile([128, CC], F32, tag="tgw")
        tgb = res.tile([128, CC], F32, tag="tgb")
        tgA = res.tile([128, 8], F16, tag="tgA")
        tgB = res.tile([8, 128], F16, tag="tgB")
        teye = res.tile([128, 128], F16, tag="teye")
        eps = res.tile([8, 1], F32, tag="eps")
        alpha = res.tile([128, CC], F32, tag="alpha")      # per-channel GN scale
        beta = res.tile([128, CC], F32, tag="beta")        # per-channel GN shift
        # raw (non-pool) SBUF tensor: fixed physical address so the raw
        # post-Tile epilogue DMA can reference it
        ost_full = nc.alloc_sbuf_tensor("ost_full", [128, CC, NQ], F32).ap()
        for t, d in ((twq, wq), (twk, wk), (twv, wv), (twp, wp), (tbq, bq),
                     (tbk, bk), (tbp, bp), (tgw, gw), (tgb, gb), (tgA, gA),
                     (tgB, gB), (teye, eye)):
            nc.sync.dma_start(out=t, in_=d[:])
        nc.vector.memset(eps, 1e-5)

        xhip = tc.tile_pool(name="xhip", bufs=1)
        xhi_pool = xhip.__enter__()
        xhi = xhi_pool.tile([128, CC, NQ], F32, tag="xhi")   # x cols [2048,4096)

        def xslice(s):
            """x slice [128, CC, 512] for n-slice s of 8 (SBUF-resident)."""
            if s < 4:
                return xlo[:, :, s * 512:(s + 1) * 512]
            return xhi[:, :, (s - 4) * 512:(s - 3) * 512]

        # load x once: unique-range DMAs into fresh tiles -> zero waits each
        for cc in range(CC):
            for s in range(8):
                nc.sync.dma_start(out=xslice(s)[:, cc, :],
                                  in_=xv[:, cc, s * 512:(s + 1) * 512])

        # ================= Phase A: GroupNorm stats =================
        mmp_cm = tc.tile_pool(name="mmp", bufs=2, space="PSUM")
        mmp = mmp_cm.__enter__()
        with tc.tile_pool(name="gnp", bufs=2) as gnp, \
             tc.tile_pool(name="gns", bufs=1) as gns:
            me = gns.tile([128, CC, 2], F16, tag="me")    # [mean, E[x^2]-1] fp16
            rs = gns.tile([8, CC, 2], F16, tag="rs")      # [mean_g, rstd-1] fp16
            bc = gns.tile([128, CC, 2], F32, tag="bc")    # broadcast back
            for cc in range(CC):
                st = gnp.tile([128, 8, 6], F32, tag="st")
                for s in range(8):
                    nc.vector.bn_stats(out=st[:, s, :], in_=xslice(s)[:, cc, :])
                mv = gnp.tile([128, 2], F32, tag="mv")
                nc.vector.bn_aggr(out=mv, in_=st)
                # me = [mean, var + mean^2]
                nc.vector.tensor_copy(me[:, cc, 0:1], mv[:, 0:1])
                sq = gnp.tile([128, 1], F32, tag="sq")
                nc.vector.tensor_mul(sq, mv[:, 0:1], mv[:, 0:1])
                e2 = gnp.tile([128, 1], F32, tag="e2")
                nc.vector.tensor_add(e2, mv[:, 1:2], sq)
                nc.vector.tensor_scalar_add(out=me[:, cc, 1:2], in0=e2, scalar1=-1.0)
            for cc in range(CC):
                gp = mmp.tile([8, 2], F32, tag="mm")
                nc.tensor.matmul(gp, tgA, me[:, cc, :], start=True, stop=True)
                gg = gns.tile([8, 2], F32, tag="gg")
                nc.vector.tensor_copy(gg, gp)
                nc.vector.tensor_scalar_add(out=gg[:, 1:2], in0=gg[:, 1:2], scalar1=1.0)
                # mean_g at [:,0], E[x^2]_g at [:,1] -> rstd
                m2 = gns.tile([8, 1], F32, tag="m2")
                nc.vector.tensor_mul(m2, gg[:, 0:1], gg[:, 0:1])
                var = gns.tile([8, 1], F32, tag="var")
                nc.vector.tensor_sub(var, gg[:, 1:2], m2)
                sd = gns.tile([8, 1], F32, tag="sd")
                nc.scalar.activation(out=sd, in_=var, func=AF.Sqrt, bias=eps, scale=1.0)
                nc.vector.tensor_copy(rs[:, cc, 0:1], gg[:, 0:1])
                rst = gns.tile([8, 1], F32, tag="rst")
                nc.vector.reciprocal(rst, sd)
                nc.vector.tensor_scalar_add(out=rs[:, cc, 1:2], in0=rst, scalar1=-1.0)
            for cc in range(CC):
                bp2 = mmp.tile([128, 2], F32, tag="mm")
                nc.tensor.matmul(bp2, tgB, rs[:, cc, :], start=True, stop=True)
                nc.vector.tensor_copy(bc[:, cc, :], bp2)
                nc.vector.tensor_scalar_add(out=bc[:, cc, 1:2], in0=bc[:, cc, 1:2], scalar1=1.0)
                # alpha = rstd * gn_w ; beta = gn_b - mean * alpha
                nc.vector.tensor_mul(alpha[:, cc:cc + 1], bc[:, cc, 1:2], tgw[:, cc:cc + 1])
                tm = gns.tile([128, 1], F32, tag="tm")
                nc.vector.tensor_mul(tm, bc[:, cc, 0:1], alpha[:, cc:cc + 1])
                nc.vector.tensor_sub(beta[:, cc:cc + 1], tgb[:, cc:cc + 1], tm)

        # ================= Phase B: normalize + q/k/vT convs =================
        with tc.tile_pool(name="cvh", bufs=3) as cvh:
            for s in range(8):                      # n-slices of 512
                hs = cvh.tile([128, CC, 512], F16, tag="hs")
                for cc in range(CC):
                    nc.vector.tensor_scalar(
                        out=hs[:, cc, :], in0=xslice(s)[:, cc, :],
                        scalar1=alpha[:, cc:cc + 1], scalar2=beta[:, cc:cc + 1],
                        op0=mybir.AluOpType.mult, op1=mybir.AluOpType.add)
                for oc in range(CC):                # k conv
                    ps = mmp.tile([128, 512], F32, tag="mm")
                    for cc in range(CC):
                        nc.tensor.matmul(ps, twk[:, cc, oc * 128:(oc + 1) * 128],
                                         hs[:, cc, :], start=(cc == 0), stop=(cc == CC - 1))
                    nc.vector.tensor_scalar_add(out=kt[:, oc, s * 512:(s + 1) * 512],
                                                in0=ps, scalar1=tbk[:, oc:oc + 1])
                for nt in range(4):                 # vT conv
                    ps = mmp.tile([128, 512], F32, tag="mm")
                    for cc in range(CC):
                        nc.tensor.matmul(ps, hs[:, cc, nt * 128:(nt + 1) * 128],
                                         twv[:, cc, :], start=(cc == 0), stop=(cc == CC - 1))
                    nc.vector.tensor_copy(vt[:, s * 4 + nt, :], ps)
                if s < 4:                           # q conv (first half only)
                    for oc in range(CC):
                        ps = mmp.tile([128, 512], F32, tag="mm")
                        for cc in range(CC):
                            nc.tensor.matmul(ps, twq[:, cc, oc * 128:(oc + 1) * 128],
                                             hs[:, cc, :], start=(cc == 0), stop=(cc == CC - 1))
                        nc.scalar.activation(out=qt[:, oc, s * 512:(s + 1) * 512], in_=ps,
                                             func=AF.Identity, bias=tbq[:, oc:oc + 1], scale=1.0)

        xhip.__exit__(None, None, None)                    # free xhi before Phase C
        mmp_cm.__exit__(None, None, None)                  # free conv psum banks
        trp = ctx.enter_context(tc.tile_pool(name="trp", bufs=2, space="PSUM"))
        prp = ctx.enter_context(tc.tile_pool(name="prp", bufs=1, space="PSUM"))

        # ================= Phase C: attention =================
        with tc.tile_pool(name="att", bufs=2) as att, \
             tc.tile_pool(name="ats", bufs=2) as ats, \
             tc.tile_pool(name="hatp", bufs=2) as hatp:
            hat = None
            for t in range(NQ // 128):              # 16 query tiles
                g, ti = t // 4, t % 4
                pt = att.tile([128, HW], F16, tag="p")
                mt = ats.tile([128, 4], F32, tag="mt")      # quarter -maxes
                sm = ats.tile([128, 4], F32, tag="sm")      # quarter exp-sums
                for qtr in range(4):
                    sc = scp.tile([128, 1024], F32, tag="sc")
                    for h2 in range(2):
                        for cc in range(CC):
                            nc.tensor.matmul(
                                sc[:, h2 * 512:(h2 + 1) * 512],
                                qt[:, cc, t * 128:(t + 1) * 128],
                                kt[:, cc, qtr * 1024 + h2 * 512: qtr * 1024 + (h2 + 1) * 512],
                                start=(cc == 0), stop=(cc == CC - 1))
                    # negated quarter max, then p = exp(s - m), rowsum
                    nc.vector.reduce_max(out=mt[:, qtr:qtr + 1], in_=sc, axis=AX, negate=True)
                    nc.scalar.activation(out=pt[:, qtr * 1024:(qtr + 1) * 1024], in_=sc,
                                         func=AF.Exp, bias=mt[:, qtr:qtr + 1], scale=1.0,
                                         accum_out=sm[:, qtr:qtr + 1])
                # combine quarters: mt holds -m_i; negM = min(-m_i) = -max(m_i)
                negM = ats.tile([128, 1], F32, tag="negM")
                nc.vector.tensor_reduce(out=negM, in_=mt, axis=AX, op=mybir.AluOpType.min)
                al = ats.tile([128, 4], F32, tag="al")      # exp(m_i - M)
                nc.scalar.activation(out=al, in_=mt, func=AF.Exp, bias=negM, scale=-1.0)
                ws = ats.tile([128, 4], F32, tag="ws")
                nc.vector.tensor_mul(ws, al, sm)
                dd = ats.tile([128, 1], F32, tag="dd")
                nc.vector.reduce_sum(out=dd, in_=ws, axis=AX)
                rd = ats.tile([128, 1], F32, tag="rd")
                nc.vector.reciprocal(rd, dd)
                scl = ats.tile([128, 4], F32, tag="scl")
                nc.vector.tensor_scalar_mul(out=scl, in0=al, scalar1=rd)
                # per-quarter diagonal scale matrices: D_i = diag(scl[:, i]).
                # The p-transpose becomes a regular matmul p_chunk.T @ D_i,
                # fusing softmax normalization into the transpose for free.
                Dt = ats.tile([128, 4, 128], F16, tag="Dt")
                for i in range(4):
                    nc.vector.tensor_scalar_mul(out=Dt[:, i, :], in0=teye,
                                                scalar1=scl[:, i:i + 1])
                # transpose+scale p -> pT [j, q]
                pT = att.tile([128, HW // 128, 128], F16, tag="pT")
                for r in range(16):
                    tp = trp.tile([128, 2, 128], F32, tag="tr")
                    for i in range(2):
                        jc = 2 * r + i
                        nc.tensor.matmul(tp[:, i, :], pt[:, jc * 128:(jc + 1) * 128],
                                         Dt[:, jc // 8, :],
                                         start=(i == 0), stop=(i == 1))
                    nc.scalar.copy(out=pT[:, 2 * r:2 * r + 2, :], in_=tp)
                # h_attT[q, c] = sum_j pT[j, q]^T vT[j, c]
                av = avp.tile([128, C], F32, tag="av")
                for jc in range(HW // 128):
                    nc.tensor.matmul(av, pT[:, jc, :], vt[:, jc, :],
                                     start=(jc == 0), stop=(jc == HW // 128 - 1))
                hts = ats.tile([128, C], F16, tag="hts")
                nc.scalar.copy(out=hts, in_=av)
                # transpose h_attT -> h_att[c, q] into group tile
                if ti == 0:
                    hat = hatp.tile([128, CC, 512], F16, tag="hat")
                th = trp.tile([128, 4, 128], F16, tag="tr")
                for cc in range(CC):
                    nc.tensor.matmul(th[:, cc, :], hts[:, cc * 128:(cc + 1) * 128],
                                     teye, is_transpose=True,
                                     start=(cc == 0), stop=(cc == CC - 1))
                nc.scalar.copy(out=hat[:, :, ti * 128:(ti + 1) * 128], in_=th)
                if ti == 3:                          # proj + residual for group g
                    for oc in range(CC):
                        pp = prp.tile([128, 512], F32, tag="pr")
                        for cc in range(CC):
                            nc.tensor.matmul(pp, twp[:, cc, oc * 128:(oc + 1) * 128],
                                             hat[:, cc, :], start=(cc == 0), stop=(cc == CC - 1))
                        sl = ost_full[:, oc, g * 512:(g + 1) * 512]
                        nc.vector.tensor_scalar_add(out=sl, in0=pp,
                                                    scalar1=tbp[:, oc:oc + 1])
                        nc.vector.tensor_add(sl, sl, xlo[:, oc, g * 512:(g + 1) * 512])
    return ost_full


def prep_inputs(x, gn_w, gn_b, q_w, q_b, k_w, k_b, v_w, v_b, p_w, p_b):
    """Host-side prep shared across cores. Returns dict of np arrays."""
    s = 1.0 / np.sqrt(C)

    def wT(w):  # [O,C] -> lhsT layout [p, cc, O]; tile[c', o] = w[o, c']
        return np.ascontiguousarray(
            w.T.reshape(CC, 128, C).transpose(1, 0, 2)).astype(np.float16)

    def vec(b):  # [C] -> [p, cc]
        return np.ascontiguousarray(b.reshape(CC, 128).T).astype(np.float32)

    gA = np.zeros((128, 8), np.float32)
    for p in range(128):
        gA[p, p // 16] = 1.0 / 16.0
    gB = np.zeros((8, 128), np.float32)
    for p in range(128):
        gB[p // 16, p] = 1.0
    bp_eff = p_b + p_w @ v_b
    return {
        "wq": wT(q_w * s), "wk": wT(k_w), "wv": wT(v_w), "wp": wT(p_w),
        "bq": vec(q_b * s), "bk": vec(k_b), "bp": vec(bp_eff),
        "gw": vec(gn_w), "gb": vec(gn_b), "gA": gA.astype(np.float16), "gB": gB.astype(np.float16),
        "eye": np.eye(128, dtype=np.float16),
    }


_CACHED = {}


def kernel(x, gn_w, gn_b, q_w, q_b, k_w, k_b, v_w, v_b, p_w, p_b):
    from concourse.bass_utils import run_bass_kernel_spmd

    x = np.asarray(x, np.float32)
    args = [np.asarray(a, np.float32) for a in
            (gn_w, gn_b, q_w, q_b, k_w, k_b, v_w, v_b, p_w, p_b)]
    common = prep_inputs(x, *args)

    if "nc" not in _CACHED:
        _CACHED["nc"] = build_kernel()
    nc = _CACHED["nc"]

    xf = x.reshape(B, C, HW)
    in_maps = []
    for core in range(8):
        b, half = core // 2, core % 2
        xb = xf[b]
        if half == 1:
            xb = np.concatenate([xb[:, NQ:], xb[:, :NQ]], axis=1)
        m = dict(common)
        m["xb"] = np.ascontiguousarray(xb)
        in_maps.append(m)

    res = run_bass_kernel_spmd(nc, in_maps, core_ids=list(range(8)))
    _CACHED["last_res"] = res
    outf = np.empty((B, C, HW), np.float32)
    for core in range(8):
        b, half = core // 2, core % 2
        outf[b][:, half * NQ:(half + 1) * NQ] = res.results[core]["out"]
    return outf.reshape(B, C, 64, 64)


if __name__ == "__main__":
    nc = build_kernel()
    print("built ok")



# revision 21
# speedup vs baseline: 2.6667x; 2.6667x over previous
"""AttnBlock (GroupNorm + single-head spatial attention + residual) on 8 TRN2 cores.

Sharding: core i handles batch b=i//2, query-half h=i%2 (2048 of 4096 spatial
positions). Keys/values span all 4096 positions, computed per-core from the
same batch input - no collectives. The host permutes each core's input so its
query half is always columns [0,2048).

v2 design (fp8 DoubleRow everywhere):
- All heavy matmuls run fp8e4 with MatmulPerfMode.DoubleRow (2 K-tiles of 128
  packed per instruction, 0.5 cycles/row = 4x fp16 MAC throughput).
- Scores computed TRANSPOSED: scT[j,i] = sum_c k[c,j] (q+bq)[c,i]. Softmax j
  runs over the partition dim, so exp needs no transpose matmuls at all:
  p^T = exp(scale*scT - 2) on the scalar engine (constant bias is
  softmax-invariant; keeps exp in fp8e4 range). k-bias cancels in softmax and
  is dropped; q-bias is folded into the q-conv psum->sbuf copy.
- Softmax denominator: DoubleRow matmul of pT-slices against a ones RHS gives
  den[i] per-partition; normalization fuses into the ACT psum->sbuf copy via
  per-partition scale=1/den.
- GroupNorm is folded into the conv weights on device (W' = W * alpha[c]); x
  is sent once as fp8 (conv rhs) - the beta terms become per-channel biases:
  q gets Wq@beta+bq in its copy, k needs none (softmax), v's Wv@beta flows
  through attention unchanged (sum p = 1) and is added to the residual via
  Wp@(Wv@beta); host pre-adds p_b + p_w@v_b into the residual input xr.
- Residual+bias enter the proj PSUM through an identity matmul of xr (bf16),
  and the output is DMA'd PSUM->DRAM directly.

DMA discipline: every DMA descriptor has exactly ONE wait slot; loads target
fresh resident tiles (zero waits), stores carry a single PE wait.
"""
import sys

for p in ("/opt/trn_rl_repo",):
    if p not in sys.path:
        sys.path.insert(0, p)

import numpy as np

import concourse.bass as bass
import concourse.mybir as mybir
import concourse.tile as tile

B, C, HW = 4, 512, 4096
NQ = HW // 2           # query positions per core
CC = C // 128          # channel chunks
JT = HW // 128         # j (key) tiles
F32 = mybir.dt.float32
F16 = mybir.dt.float16
BF16 = mybir.dt.bfloat16
F8 = mybir.dt.float8e4
AX = mybir.AxisListType.X
AF = mybir.ActivationFunctionType
DR = mybir.MatmulPerfMode.DoubleRow
SCL = 1.0 / np.sqrt(C)      # softmax scale, applied inside exp
EBIAS = -2.0                # constant exp bias (softmax-invariant)


def build_kernel():
    nc = bass.Bass()
    xf8 = nc.dram_tensor("xf8", [128, CC, HW], F8, kind="ExternalInput")
    xr = nc.dram_tensor("xr", [128, CC, NQ], BF16, kind="ExternalInput")
    w16q = nc.dram_tensor("w16q", [128, CC, C], F16, kind="ExternalInput")
    w16k = nc.dram_tensor("w16k", [128, CC, C], F16, kind="ExternalInput")
    w16v = nc.dram_tensor("w16v", [128, CC, C], F16, kind="ExternalInput")
    wp8 = nc.dram_tensor("wp8", [128, CC, C], F8, kind="ExternalInput")
    bq = nc.dram_tensor("bq", [128, CC], F32, kind="ExternalInput")
    gw = nc.dram_tensor("gw", [128, CC], F32, kind="ExternalInput")
    gb = nc.dram_tensor("gb", [128, CC], F32, kind="ExternalInput")
    gA = nc.dram_tensor("gA", [128, 8], F16, kind="ExternalInput")
    gB = nc.dram_tensor("gB", [8, 128], F16, kind="ExternalInput")
    eye16 = nc.dram_tensor("eye16", [128, 128], F16, kind="ExternalInput")
    ones8 = nc.dram_tensor("ones8", [128, 2, 1], F8, kind="ExternalInput")
    out = nc.dram_tensor("out", [C, NQ], F32, kind="ExternalOutput")

    ov = out.rearrange("(cc p) n -> p cc n", p=128)     # [128, CC, NQ]

    fin = nc.dram_tensor("fin", [1, 4], F32, kind="ExternalOutput")
    mark = nc.alloc_sbuf_tensor("mark", [1, 4], F32).ap()
    end_sem = nc.alloc_semaphore("end_sem")
    with tile.TileContext(nc) as tc:
        build_body(nc, tc, xf8, xr, w16q, w16k, w16v, wp8, bq, gw, gb,
                   gA, gB, eye16, ones8, ov, mark)
    _legalize_waits(nc)
    # Epilogue: HWDGE DMAs complete in FIFO order per engine ring, so a marker
    # DMA issued after all in-tc output stores lands only once they have.
    fin_sem = nc.alloc_semaphore("fin_sem", num=next(nc._free_sem_ids))
    nc.sync.dma_start(out=fin[:], in_=mark).then_inc(fin_sem, 16)
    nc.sync.wait_ge(fin_sem, 16).then_inc(end_sem, 1)
    return nc


def _legalize_waits(nc):
    """Walrus codegen allows ONE sync wait per ISA instruction (TPB_EVENTS has a
    single wait slot). Tile can emit several (same-engine pipeline hazard +
    cross-engine deps). Split: keep one wait on the instruction, move the rest
    onto engine NoOps inserted immediately before it (same engine queue)."""
    import bass_rust as _br
    used = set()
    for fn in nc.m.functions:
        for blk in fn.blocks:
            for inst in blk.instructions:
                si = inst.sync_info
                if si is not None:
                    for e in list(si.on_wait or []) + list(si.on_update or []):
                        used.add(e.id)
    free_ids = (i for i in range(254, 0, -1) if i not in used)
    nc._free_sem_ids = free_ids
    legal_sems = {}
    for fn in nc.m.functions:
        for blk in fn.blocks:
            out = []
            for inst in blk.instructions:
                si = inst.sync_info
                waits = list(si.on_wait) if si is not None and si.on_wait else []
                if len(waits) > 1:
                    if isinstance(inst, mybir.InstDMACopy):
                        # DMAs dispatch through the (in-order) SP sequencer:
                        # hoist extra waits onto SP event-sem instructions
                        # emitted just before the DMA.
                        for w in waits[:-1]:
                            ev = mybir.InstEventSemaphore(
                                name=nc.get_next_instruction_name(),
                                engine=inst.engine,
                                ins=[], outs=[],
                                sync_info=mybir.SyncInfo(on_wait=[w],
                                                         on_update=[]),
                            )
                            if inst.engine not in legal_sems:
                                legal_sems[inst.engine] = nc.alloc_semaphore(
                                    f"legalize_sem_{inst.engine}",
                                    num=next(free_ids))
                            _br.then_inc(ev, legal_sems[inst.engine], 1, False)
                            out.append(ev)
                        inst.sync_info = mybir.SyncInfo(
                            on_wait=[waits[-1]],
                            on_update=list(si.on_update or []))
                        out.append(inst)
                        continue
                    for w in waits[:-1]:
                        nop = mybir.InstNoOp(
                            name=nc.get_next_instruction_name(),
                            engine=inst.engine,
                            bass_nofuse=True,
                            sync_info=mybir.SyncInfo(on_wait=[w], on_update=[]),
                        )
                        if inst.engine not in legal_sems:
                            legal_sems[inst.engine] = nc.alloc_semaphore(
                                f"legalize_sem_{inst.engine}", num=next(free_ids))
                        _br.then_inc(nop, legal_sems[inst.engine], 1, False)
                        out.append(nop)
                    inst.sync_info = mybir.SyncInfo(
                        on_wait=[waits[-1]], on_update=list(si.on_update or []))
                out.append(inst)
            blk.instructions = out


def build_body(nc, tc, xf8, xr, w16q, w16k, w16v, wp8, bq, gw, gb,
               gA, gB, eye16, ones8, ov, mark):
    import contextlib

    ctx = contextlib.ExitStack()
    with ctx:
        res = ctx.enter_context(tc.tile_pool(name="res", bufs=1))     # resident

        # --- resident tensors (inputs) ---
        txf = res.tile([128, CC, HW], F8, tag="txf")        # x fp8 [c,cc,n]
        txr = res.tile([128, CC, NQ], BF16, tag="txr")      # x + proj-bias (bf16)
        tw16q = res.tile([128, CC, C], F16, tag="tw16q")
        tw16k = res.tile([128, CC, C], F16, tag="tw16k")
        tw16v = res.tile([128, CC, C], F16, tag="tw16v")
        twp8 = res.tile([128, CC, C], F8, tag="twp8")
        tbq = res.tile([128, CC], F32, tag="tbq")
        tgw = res.tile([128, CC], F32, tag="tgw")
        tgb = res.tile([128, CC], F32, tag="tgb")
        tgA = res.tile([128, 8], F16, tag="tgA")
        tgB = res.tile([8, 128], F16, tag="tgB")
        teye16 = res.tile([128, 128], F16, tag="teye16")
        tones8 = res.tile([128, 2, 1], F8, tag="tones8")
        eps = res.tile([8, 1], F32, tag="eps")
        mneg2 = res.tile([128, 1], F32, tag="mneg2")

        # --- resident tensors (device-produced) ---
        w8q = res.tile([128, CC, C], F8, tag="w8q")         # alpha-scaled conv w
        w8k = res.tile([128, CC, C], F8, tag="w8k")
        w8v = res.tile([128, CC, C], F8, tag="w8v")
        alpha = res.tile([128, CC], F32, tag="alpha")
        beta = res.tile([128, CC], F32, tag="beta")
        betah = res.tile([128, CC], F16, tag="betah")
        qbias = res.tile([128, CC], F32, tag="qbias")       # Wq@beta + bq
        cv8 = res.tile([128, CC], F8, tag="cv8")            # Wv@beta
        t2 = res.tile([128, CC], F32, tag="t2")             # Wp@cv
        kt = res.tile([128, CC, HW], F8, tag="kt")          # k[c,j]
        vt = res.tile([128, JT, C], F8, tag="vt")           # vT[j,c]
        qt = res.tile([128, CC, NQ], F8, tag="qt")          # (q+bq)[c,i]
        pT = res.tile([128, JT, NQ], F8, tag="pT")          # exp(scores)T [j,i]

        # x per-cc loads: each DMA targets a unique fresh range -> zero waits
        for cc in range(CC):
            nc.sync.dma_start(out=txf[:, cc, :], in_=xf8[:, cc, :])
        for t, d in ((txr, xr), (tw16q, w16q), (tw16k, w16k), (tw16v, w16v),
                     (twp8, wp8), (tbq, bq), (tgw, gw), (tgb, gb), (tgA, gA),
                     (tgB, gB), (teye16, eye16), (tones8, ones8)):
            nc.sync.dma_start(out=t, in_=d[:])
        nc.vector.memset(eps, 1e-5)
        nc.vector.memset(mark, 0.0)
        nc.vector.memset(mneg2, EBIAS)

        # ================= Stage 0: GroupNorm stats -> alpha/beta ===========
        mm0_cm = tc.tile_pool(name="mm0", bufs=2, space="PSUM")
        mm0 = mm0_cm.__enter__()
        with tc.tile_pool(name="gnp", bufs=2) as gnp, \
             tc.tile_pool(name="gns", bufs=1) as gns:
            mvall = gns.tile([128, CC, 2], F32, tag="mvall")
            me = gns.tile([128, CC, 2], F16, tag="me")    # [mean, E[x^2]-1]
            for cc in range(CC):
                st = gnp.tile([128, 8, 6], F32, tag="st")
                for s in range(8):
                    nc.vector.bn_stats(out=st[:, s, :],
                                       in_=txf[:, cc, s * 512:(s + 1) * 512])
                nc.vector.bn_aggr(out=mvall[:, cc, :], in_=st)
            sq = gns.tile([128, CC], F32, tag="sq")
            nc.vector.tensor_mul(sq, mvall[:, :, 0], mvall[:, :, 0])
            nc.vector.tensor_copy(me[:, :, 0], mvall[:, :, 0])
            e2 = gns.tile([128, CC], F32, tag="e2")
            nc.vector.tensor_add(e2, mvall[:, :, 1], sq)
            nc.vector.tensor_scalar_add(out=me[:, :, 1], in0=e2, scalar1=-1.0)
            # group reduce: [128 ch, (cc,2)] -> [8 grp, (cc,2)]
            gp = mm0.tile([8, CC, 2], F32, tag="mm")
            nc.tensor.matmul(gp, tgA, me, start=True, stop=True)
            gg = gns.tile([8, CC, 2], F32, tag="gg")
            nc.vector.tensor_copy(gg, gp)
            nc.vector.tensor_scalar_add(out=gg[:, :, 1], in0=gg[:, :, 1],
                                        scalar1=1.0)
            m2 = gns.tile([8, CC], F32, tag="m2")
            nc.vector.tensor_mul(m2, gg[:, :, 0], gg[:, :, 0])
            var = gns.tile([8, CC], F32, tag="var")
            nc.vector.tensor_sub(var, gg[:, :, 1], m2)
            sd = gns.tile([8, CC], F32, tag="sd")
            nc.scalar.activation(out=sd, in_=var, func=AF.Sqrt, bias=eps,
                                 scale=1.0)
            rst = gns.tile([8, CC], F32, tag="rst")
            nc.vector.reciprocal(rst, sd)
            rs = gns.tile([8, CC, 2], F16, tag="rs")      # [mean_g, rstd-1]
            nc.vector.tensor_copy(rs[:, :, 0], gg[:, :, 0])
            nc.vector.tensor_scalar_add(out=rs[:, :, 1], in0=rst, scalar1=-1.0)
            # broadcast back to channels
            bp2 = mm0.tile([128, CC, 2], F32, tag="mm")
            nc.tensor.matmul(bp2, tgB, rs, start=True, stop=True)
            bc = gns.tile([128, CC, 2], F32, tag="bc")
            nc.vector.tensor_copy(bc, bp2)
            nc.vector.tensor_scalar_add(out=bc[:, :, 1], in0=bc[:, :, 1],
                                        scalar1=1.0)
            # alpha = rstd * gn_w ; beta = gn_b - mean * alpha
            nc.vector.tensor_mul(alpha, bc[:, :, 1], tgw)
            tm = gns.tile([128, CC], F32, tag="tm")
            nc.vector.tensor_mul(tm, bc[:, :, 0], alpha)
            nc.vector.tensor_sub(beta, tgb, tm)
            nc.vector.tensor_copy(betah, beta)

        # fold GN into weights: w8 = w16 * alpha[c]  (split DVE / Pool)
        for (w16, w8) in ((tw16q, w8q), (tw16k, w8k), (tw16v, w8v)):
            for cc in range(CC):
                nc.vector.tensor_scalar_mul(out=w8[:, cc, :], in0=w16[:, cc, :],
                                            scalar1=alpha[:, cc:cc + 1])
        # bias terms: qb = Wq@beta (+bq), cv = Wv@beta, t2 = Wp@cv
        qb_ps = mm0.tile([128, CC], F32, tag="mm")
        vb_ps = mm0.tile([128, CC], F32, tag="mm")
        for ps, w16 in ((qb_ps, tw16q), (vb_ps, tw16v)):
            n = 0
            for oc in range(CC):
                for cc in range(CC):
                    nc.tensor.matmul(ps[:, oc:oc + 1],
                                     w16[:, cc, oc * 128:(oc + 1) * 128],
                                     betah[:, cc:cc + 1],
                                     start=(n == 0), stop=(n == CC * CC - 1))
                    n += 1
        nc.vector.tensor_add(qbias, qb_ps, tbq)
        nc.vector.tensor_copy(cv8, vb_ps)
        t2_ps = mm0.tile([128, CC], F32, tag="mm")
        n = 0
        for oc in range(CC):
            for cc in range(CC):
                nc.tensor.matmul(t2_ps[:, oc:oc + 1],
                                 twp8[:, cc, oc * 128:(oc + 1) * 128],
                                 cv8[:, cc:cc + 1],
                                 start=(n == 0), stop=(n == CC * CC - 1))
                n += 1
        nc.vector.tensor_copy(t2, t2_ps)
        # residual correction: xr += Wp@Wv@beta (per out-channel constant)
        for cc in range(CC):
            nc.gpsimd.tensor_scalar_add(out=txr[:, cc, :], in0=txr[:, cc, :],
                                        scalar1=t2[:, cc:cc + 1])
        mm0_cm.__exit__(None, None, None)

        # ================= Stage 1: convs + scores + exp ====================
        cvp_cm = tc.tile_pool(name="cvp", bufs=2, space="PSUM")
        scp_cm = tc.tile_pool(name="scp", bufs=2, space="PSUM")
        cvp = cvp_cm.__enter__()
        scp = scp_cm.__enter__()

        def conv_pair(w8, oc0, nslice, dst, bias):
            """Two [128,512] conv chunks (oc0, oc0+1) in one 2-bank psum tile.
            GPSIMD cannot touch PSUM on hw, so copies run on DVE; pairing
            chunks into one [128,1024] copy amortizes the per-op overhead."""
            ps = cvp.tile([128, 2, 512], F32, tag="cv")
            for g in range(2):
                osl = slice((oc0 + g) * 128, (oc0 + g + 1) * 128)
                nc.tensor.matmul(ps[:, g, :], w8[:, 0:2, osl], txf[:, 0:2, nslice],
                                 start=True, stop=False, perf_mode=DR)
                nc.tensor.matmul(ps[:, g, :], w8[:, 2:4, osl], txf[:, 2:4, nslice],
                                 start=False, stop=True, perf_mode=DR)
            if bias is None:
                nc.vector.tensor_copy(dst, ps)
            else:
                for g in range(2):
                    nc.vector.tensor_scalar_add(out=dst[g], in0=ps[:, g, :],
                                                scalar1=bias[g])

        def vconv_pair(jt0, dst):
            """vT for j-tiles (jt0, jt0+1) -> one [128,2,C] copy."""
            ps = cvp.tile([128, 2, 512], F32, tag="cv")
            for g in range(2):
                js = slice((jt0 + g) * 128, (jt0 + g + 1) * 128)
                nc.tensor.matmul(ps[:, g, :], txf[:, 0:2, js], w8v[:, 0:2, :],
                                 start=True, stop=False, perf_mode=DR)
                nc.tensor.matmul(ps[:, g, :], txf[:, 2:4, js], w8v[:, 2:4, :],
                                 start=False, stop=True, perf_mode=DR)
            nc.vector.tensor_copy(dst, ps)

        def scores_jt(jt):
            """scT[j in jt, all i] + exp -> pT[:, jt, :], two 1024-wide halves."""
            js = slice(jt * 128, (jt + 1) * 128)
            for h in range(2):
                sc = scp.tile([128, 2, 512], F32, tag="sc")
                for g in range(2):
                    ig = 2 * h + g
                    isl = slice(ig * 512, (ig + 1) * 512)
                    nc.tensor.matmul(sc[:, g, :], kt[:, 0:2, js], qt[:, 0:2, isl],
                                     start=True, stop=False, perf_mode=DR)
                    nc.tensor.matmul(sc[:, g, :], kt[:, 2:4, js], qt[:, 2:4, isl],
                                     start=False, stop=True, perf_mode=DR)
                nc.scalar.activation(out=pT[:, jt, h * 1024:(h + 1) * 1024],
                                     in_=sc, func=AF.Exp, bias=mneg2, scale=SCL)

        # q conv (queries = n cols [0,2048))
        for s in range(4):
            nsl = slice(s * 512, (s + 1) * 512)
            for oc0 in (0, 2):
                conv_pair(w8q, oc0, nsl,
                          [qt[:, oc0, nsl], qt[:, oc0 + 1, nsl]],
                          [qbias[:, oc0:oc0 + 1], qbias[:, oc0 + 1:oc0 + 2]])
        # k/v convs per n-slice, scores trail one slice behind (avoid PE HOL stall)
        for s in range(8):
            nsl = slice(s * 512, (s + 1) * 512)
            for oc0 in (0, 2):
                conv_pair(w8k, oc0, nsl, kt[:, oc0:oc0 + 2, nsl], None)
            for j0 in (0, 2):
                vconv_pair(4 * s + j0, vt[:, 4 * s + j0:4 * s + j0 + 2, :])
            if s >= 1:
                for j4 in range(4):
                    scores_jt(4 * (s - 1) + j4)
        for j4 in range(4):
            scores_jt(28 + j4)

        scp_cm.__exit__(None, None, None)
        cvp_cm.__exit__(None, None, None)

        # ================= Stage 2: attention out ===========================
        avp = ctx.enter_context(tc.tile_pool(name="avp", bufs=2, space="PSUM"))
        dnp = ctx.enter_context(tc.tile_pool(name="dnp", bufs=2, space="PSUM"))
        thp = ctx.enter_context(tc.tile_pool(name="thp", bufs=2, space="PSUM"))
        ppp = ctx.enter_context(tc.tile_pool(name="ppp", bufs=2, space="PSUM"))
        with tc.tile_pool(name="hats", bufs=2) as hatp, \
             tc.tile_pool(name="htsp", bufs=2) as htsp, \
             tc.tile_pool(name="ostp", bufs=4) as ostp, \
             tc.tile_pool(name="rdp", bufs=2) as rdp:
            for ig in range(4):
                hat = hatp.tile([128, CC, 512], F8, tag="hat")
                for i4 in range(4):
                    it = 4 * ig + i4
                    isl = slice(it * 128, (it + 1) * 128)
                    av = avp.tile([128, C], F32, tag="av")
                    den = dnp.tile([128, 1], F32, tag="den")
                    for t in range(JT // 2):
                        nc.tensor.matmul(av, pT[:, 2 * t:2 * t + 2, isl],
                                         vt[:, 2 * t:2 * t + 2, :],
                                         start=(t == 0), stop=(t == JT // 2 - 1),
                                         perf_mode=DR)
                    for t in range(JT // 2):
                        nc.tensor.matmul(den, pT[:, 2 * t:2 * t + 2, isl],
                                         tones8,
                                         start=(t == 0), stop=(t == JT // 2 - 1),
                                         perf_mode=DR)
                    rd = rdp.tile([128, 1], F32, tag="rd")
                    nc.vector.reciprocal(rd, den)
                    hts = htsp.tile([128, C], F16, tag="hts")
                    nc.scalar.activation(out=hts, in_=av, func=AF.Copy,
                                         scale=rd[:, 0:1])
                    th = thp.tile([128, CC, 256], F16, tag="th")
                    for cc in range(CC):
                        nc.tensor.matmul(th[:, cc, 0:128],
                                         hts[:, cc * 128:(cc + 1) * 128],
                                         teye16, is_transpose=True,
                                         start=(cc == 0), stop=(cc == CC - 1))
                    nc.scalar.copy(out=hat[:, :, i4 * 128:(i4 + 1) * 128],
                                   in_=th[:, :, 0:128])
                # proj; residual add fuses into the psum->sbuf move
                gsl = slice(ig * 512, (ig + 1) * 512)
                for oc in range(CC):
                    pp = ppp.tile([128, 512], F32, tag="pp")
                    nc.tensor.matmul(pp, twp8[:, 0:2, oc * 128:(oc + 1) * 128],
                                     hat[:, 0:2, :], start=True, stop=False,
                                     perf_mode=DR)
                    nc.tensor.matmul(pp, twp8[:, 2:4, oc * 128:(oc + 1) * 128],
                                     hat[:, 2:4, :], start=False, stop=True,
                                     perf_mode=DR)
                    ost = ostp.tile([128, 512], F32, tag="ost")
                    nc.vector.tensor_add(ost, pp, txr[:, oc, gsl])
                    nc.sync.dma_start(out=ov[:, oc, gsl], in_=ost)


def prep_inputs(x, gn_w, gn_b, q_w, q_b, k_w, k_b, v_w, v_b, p_w, p_b):
    """Host-side prep shared across cores. Returns dict of np arrays."""
    f8 = mybir.dt.np(F8)
    bf = mybir.dt.np(BF16)

    def wT(w, dt):  # [O,C] -> lhsT layout [p, cc, O]; tile[c', o] = w[o, c']
        return np.ascontiguousarray(
            w.T.reshape(CC, 128, C).transpose(1, 0, 2)).astype(dt)

    def vec(b):  # [C] -> [p, cc]
        return np.ascontiguousarray(b.reshape(CC, 128).T).astype(np.float32)

    gA = np.zeros((128, 8), np.float32)
    for p in range(128):
        gA[p, p // 16] = 1.0 / 16.0
    gB = np.zeros((8, 128), np.float32)
    for p in range(128):
        gB[p // 16, p] = 1.0
    bp_eff = p_b + p_w @ v_b
    return {
        "w16q": wT(q_w, np.float16), "w16k": wT(k_w, np.float16),
        "w16v": wT(v_w, np.float16), "wp8": wT(p_w, f8),
        "bq": vec(q_b), "gw": vec(gn_w), "gb": vec(gn_b),
        "gA": gA.astype(np.float16), "gB": gB.astype(np.float16),
        "eye16": np.eye(128, dtype=np.float16),
        "ones8": np.ones((128, 2, 1), np.float32).astype(f8),
        "_bp_eff": bp_eff,
    }


def per_core_input(xf, core, common):
    """Per-core tensors: xf8 (fp8 x) and xr (x[:, :NQ] + bp_eff, bf16)."""
    f8 = mybir.dt.np(F8)
    bf = mybir.dt.np(BF16)
    b, half = core // 2, core % 2
    xb = xf[b]
    if half == 1:
        xb = np.concatenate([xb[:, NQ:], xb[:, :NQ]], axis=1)
    m = {k: v for k, v in common.items() if not k.startswith("_")}
    m["xf8"] = np.ascontiguousarray(
        xb.reshape(CC, 128, HW).transpose(1, 0, 2)).astype(f8)
    xrh = xb[:, :NQ] + common["_bp_eff"][:, None]
    m["xr"] = np.ascontiguousarray(
        xrh.reshape(CC, 128, NQ).transpose(1, 0, 2)).astype(bf)
    return m


_CACHED = {}


def kernel(x, gn_w, gn_b, q_w, q_b, k_w, k_b, v_w, v_b, p_w, p_b):
    from concourse.bass_utils import run_bass_kernel_spmd

    x = np.asarray(x, np.float32)
    args = [np.asarray(a, np.float32) for a in
            (gn_w, gn_b, q_w, q_b, k_w, k_b, v_w, v_b, p_w, p_b)]
    common = prep_inputs(x, *args)

    if "nc" not in _CACHED:
        _CACHED["nc"] = build_kernel()
    nc = _CACHED["nc"]

    xf = x.reshape(B, C, HW)
    in_maps = [per_core_input(xf, core, common) for core in range(8)]

    res = run_bass_kernel_spmd(nc, in_maps, core_ids=list(range(8)))
    _CACHED["last_res"] = res
    outf = np.empty((B, C, HW), np.float32)
    for core in range(8):
        b, half = core // 2, core % 2
        outf[b][:, half * NQ:(half + 1) * NQ] = res.results[core]["out"]
    return outf.reshape(B, C, 64, 64)


if __name__ == "__main__":
    nc = build_kernel()
    print("built ok")
